# revision 1
# baseline (speedup 1.0000x reference)
"""BaiChuan attention layer on 8 Trainium2 NeuronCores.

Sharding: tensor-parallel over heads within groups of 4 cores (W_pack
column-parallel, o_proj column-parallel after a per-head AllGather of
attention outputs), data-parallel over the batch across the two groups.

Per-core dataflow (core c: batch b=c//4, rank r=c%4, heads 8r..8r+8):
  stage A: qkvT[j, t] = W_core @ hs[b].T      (PE, f32r, psum-accumulated)
  stage B: per head: neox RoPE on qT,kT (DVE, swapped-half DMA loads),
           v natural layout via PE transpose, causal attention with
           s^T = kT.T-blocks @ qT (scores transposed), exp on ACT,
           softmax denominator via a ones-column matmul on PE,
           PV with p^T as moving operand, per-head AllGather of attn
           outputs (overlaps with later heads' compute).
  stage C: o_proj column-parallel over the gathered head dim, split as
           heads 0-6 (starts before the last AllGather) + head-7
           increment.  Host concatenates the m-shards.
"""
import sys
sys.path.insert(0, '/opt/trn_rl_repo')
import numpy as np
import ml_dtypes

import concourse.bass as bass
from concourse import bacc
import concourse.mybir as mybir
from concourse.tile import TileContext
from concourse.bass_utils import run_bass_kernel_spmd
from concourse.masks import make_identity
from concourse import bass_isa

f32 = mybir.dt.float32
f32r = mybir.dt.float32r
bf16 = mybir.dt.bfloat16
AF = mybir.ActivationFunctionType

B, S, H, NH = 2, 2048, 4096, 32
HD = H // NH                    # 128
THETA = 10000.0
NCORES, TPN = 8, 4              # 2 groups of 4 (DP over batch x TP over heads)
HPC = NH // TPN                 # 8 heads per core
JC = HPC * HD                   # 1024 per-core q (=k=v) width
SCALE = HD ** -0.5
GROUPS = [[0, 1, 2, 3], [4, 5, 6, 7]]
TB = 1024                       # stage-A token block
NTB = S // TB
NIB = H // 128                  # 32 contraction blocks
NJT = 3 * JC // 128             # 24 output row-tiles in stage A
NG = S // 512                   # 4 query blocks per head
NKB = S // 128                  # 16 key blocks per head
NJB = TPN * HPC                 # 32 o_proj contraction blocks
NJB_MAIN = NJB - TPN            # heads 0..6 -> jb 0..27


def build_nc():
    nc = bacc.Bacc(None)
    hsT = nc.declare_dram_parameter("hsT", [H, S], f32, isOutput=False)
    wT = nc.declare_dram_parameter("wT", [H, 3 * JC], f32, isOutput=False)
    woT = nc.declare_dram_parameter("woT", [H, JC], f32, isOutput=False)
    cosf = nc.declare_dram_parameter("cosf", [HD, S], bf16, isOutput=False)
    sinm = nc.declare_dram_parameter("sinm", [HD, S], bf16, isOutput=False)
    masks = nc.declare_dram_parameter("masks", [4, 128, 512], bf16,
                                      isOutput=False)
    out = nc.declare_dram_parameter("out", [S, JC], f32, isOutput=True)

    qkv_d = [nc.dram_tensor(f"qkv_d{j}", [128, S], f32) for j in range(NJT)]
    attn_d = [nc.dram_tensor(f"attn_d{h}", [HD, S], f32) for h in range(HPC)]
    attn_ag = nc.dram_tensor("attn_ag", [HPC - 1, TPN * HD, S], f32)
    attn_ag7 = nc.dram_tensor("attn_ag7", [TPN * HD, S], f32)

    hsT_v = hsT[:].rearrange("(n p) t -> p n t", p=128)      # [128, 32, S]
    wT_v = wT[:].rearrange("(n p) j -> p n j", p=128)        # [128, 32, 3*JC]
    woT_v = woT[:].rearrange("(n p) m -> p n m", p=128)      # [128, 32, JC]
    ag_v = attn_ag[:].rearrange("h (r p) t -> p (h r) t", p=128)  # [128, 28, S]
    ag7_v = attn_ag7[:].rearrange("(r p) t -> p r t", p=128)       # [128, 4, S]

    with TileContext(nc) as tc:
        # ---------------- stage A: fused QKV projection ----------------
        # j-tile order: per-head (v,k,q) so early heads' inputs finish first
        jt_order = []
        for hh in range(HPC):
            jt_order += [2 * HPC + hh, HPC + hh, hh]
        with nc.named_scope("stageA"), \
             tc.tile_pool(name="stA", bufs=1) as pa, \
             tc.tile_pool(name="psA", bufs=6, space="PSUM") as psA:
            for tb in range(NTB):
                hs_a = pa.tile([128, NIB // 2, TB], f32r, tag="hs_a", bufs=1,
                               name=f"hs_a_{tb}")
                hs_b = pa.tile([128, NIB // 2, TB], f32r, tag="hs_b", bufs=1,
                               name=f"hs_b_{tb}")
                for d in range(4):
                    nc.sync.dma_start(
                        out=hs_a[:, 4 * d:4 * (d + 1), :],
                        in_=hsT_v[:, 4 * d:4 * (d + 1),
                                  tb * TB:(tb + 1) * TB].bitcast(f32r))
                for d in range(4):
                    nc.sync.dma_start(
                        out=hs_b[:, 4 * d:4 * (d + 1), :],
                        in_=hsT_v[:, NIB // 2 + 4 * d:NIB // 2 + 4 * (d + 1),
                                  tb * TB:(tb + 1) * TB].bitcast(f32r))
                for jt in jt_order:
                    w_sb = pa.tile([128, NIB, 128], f32r, tag="w", bufs=4,
                                   name=f"w_{tb}_{jt}")
                    nc.sync.dma_start(
                        out=w_sb[:],
                        in_=wT_v[:, :, jt * 128:(jt + 1) * 128].bitcast(f32r))
                    for th in range(TB // 512):
                        ps = psA.tile([128, 512], f32, tag="psA",
                                      name=f"psA_{tb}_{jt}_{th}")
                        for ib in range(NIB):
                            hsrc = hs_a if ib < NIB // 2 else hs_b
                            nc.tensor.matmul(
                                ps[:], w_sb[:, ib, :],
                                hsrc[:, ib % (NIB // 2),
                                     th * 512:(th + 1) * 512],
                                start=(ib == 0), stop=(ib == NIB - 1))
                        st = pa.tile([128, 512], f32, tag="oA", bufs=4,
                                     name=f"stA_{tb}_{jt}_{th}")
                        nc.scalar.copy(st[:], ps[:])
                        nc.sync.dma_start(
                            out=qkv_d[jt][:][:, tb * TB + th * 512:
                                             tb * TB + (th + 1) * 512],
                            in_=st[:])

        # ------------- stages B+C share a right-side o_proj weight pool ------
        with tc.tile_pool(name="stWo", bufs=1, side="right") as pwo:
            wo_h0 = pwo.tile([128, NIB, JC // 2], f32r, tag="wo0", bufs=1)

            # ---------------- stage B: rope + causal attention ---------------
            with nc.named_scope("stageB"), \
                 tc.tile_pool(name="stB", bufs=1) as pb, \
                 tc.tile_pool(name="psB", bufs=1, space="PSUM") as psB:
                ident = pb.tile([128, 128], f32, tag="ident", bufs=1)
                make_identity(nc, ident[:])
                ones_f = pb.tile([128, 1], f32, tag="ones_f", bufs=1)
                nc.vector.memset(ones_f[:], 1.0)
                ones_r = pb.tile([128, 1], f32r, tag="ones_r", bufs=1)
                nc.vector.tensor_copy(ones_r[:], ones_f[:])
                cos_sb = pb.tile([128, S], bf16, tag="cos", bufs=1)
                sin_sb = pb.tile([128, S], bf16, tag="sin", bufs=1)
                nc.sync.dma_start(out=cos_sb[:], in_=cosf[:])
                nc.sync.dma_start(out=sin_sb[:], in_=sinm[:])
                mask_sb = pb.tile([128, 4, 512], bf16, tag="mask", bufs=1)
                nc.sync.dma_start(out=mask_sb[:],
                                  in_=masks[:].rearrange("v p x -> p v x"))
                for d in range(8):
                    nc.sync.dma_start(
                        out=wo_h0[:, 4 * d:4 * (d + 1), :],
                        in_=woT_v[:, 4 * d:4 * (d + 1), :JC // 2].bitcast(f32r))

                def load_rope(jt, tag, h):
                    """load qkvT_d row-block jt, apply neox rope, emit f32r"""
                    raw = pb.tile([128, S], f32, tag="raw", bufs=4,
                                  name=f"{tag}_raw_{h}")
                    nc.sync.dma_start(out=raw[:], in_=qkv_d[jt][:])
                    sw = pb.tile([128, S], f32, tag="raw", bufs=4,
                                 name=f"{tag}_sw_{h}")
                    nc.sync.dma_start(out=sw[0:64, :],
                                      in_=qkv_d[jt][:][64:128, :])
                    nc.sync.dma_start(out=sw[64:128, :],
                                      in_=qkv_d[jt][:][0:64, :])
                    t1 = pb.tile([128, S], f32, tag="ropetmp", bufs=2,
                                 name=f"{tag}_t1_{h}")
                    t2 = pb.tile([128, S], f32, tag="ropetmp", bufs=2,
                                 name=f"{tag}_t2_{h}")
                    with tc.high_priority():
                        nc.vector.tensor_mul(t1[:], raw[:], cos_sb[:])
                        nc.vector.tensor_mul(t2[:], sw[:], sin_sb[:])
                        rt = pb.tile([128, S], f32r, tag=f"{tag}_r", bufs=2,
                                     name=f"{tag}_roped_{h}")
                        nc.vector.tensor_add(rt[:], t1[:], t2[:])
                    return rt

                for h in range(HPC):
                    with nc.named_scope(f"head{h}"):
                        kT = load_rope(HPC + h, "kr", h)
                        qT = load_rope(h, "qr", h)
                        vraw = pb.tile([128, S], f32, tag="raw", bufs=4,
                                       name=f"vr_{h}")
                        nc.sync.dma_start(out=vraw[:],
                                          in_=qkv_d[2 * HPC + h][:])
                        v_sb = pb.tile([128, NKB, 128], f32r, tag="vsb",
                                       bufs=2, name=f"v_{h}")
                        for kb in range(NKB):
                            pst = psB.tile([128, 512], f32, tag="pss", bufs=4,
                                           name=f"ptr_{h}_{kb}")
                            nc.tensor.transpose(
                                pst[0:128, 0:128],
                                vraw[:, kb * 128:(kb + 1) * 128], ident[:])
                            nc.scalar.copy(v_sb[:, kb, :], pst[0:128, 0:128])

                        attn = pb.tile([128, S], f32r, tag="attn", bufs=1,
                                       name=f"attn_{h}")
                        for g in range(NG):
                            nkb = 4 * g + 4
                            po = psB.tile([128, 512], f32, tag="po", bufs=2,
                                          name=f"po_{h}_{g}")
                            pden = psB.tile([1, 512], f32, tag="pden", bufs=2,
                                            name=f"pden_{h}_{g}")
                            for kb in range(nkb):
                                pss = psB.tile([128, 512], f32, tag="pss",
                                               bufs=4, name=f"pss_{h}_{g}_{kb}")
                                nc.tensor.matmul(
                                    pss[:], kT[:, kb * 128:(kb + 1) * 128],
                                    qT[:, g * 512:(g + 1) * 512],
                                    start=True, stop=True)
                                pt = pb.tile([128, 512], f32r, tag="pt",
                                             bufs=4, name=f"pt_{h}_{g}_{kb}")
                                nc.scalar.activation(pt[:], pss[:], AF.Exp,
                                                     scale=SCALE)
                                if kb >= 4 * g:
                                    nc.vector.tensor_mul(
                                        pt[:], pt[:],
                                        mask_sb[:, kb - 4 * g, :])
                                nc.tensor.matmul(pden[:], ones_r[:], pt[:],
                                                 start=(kb == 0),
                                                 stop=(kb == nkb - 1))
                                nc.tensor.matmul(po[:], v_sb[:, kb, :], pt[:],
                                                 start=(kb == 0),
                                                 stop=(kb == nkb - 1))
                            den1 = pb.tile([1, 512], f32, tag="den1", bufs=1,
                                           name=f"den1_{h}_{g}")
                            nc.scalar.copy(den1[:], pden[:])
                            rd1 = pb.tile([1, 512], f32, tag="rd1", bufs=2,
                                          name=f"rd1_{h}_{g}")
                            nc.vector.reciprocal(rd1[:], den1[:])
                            rden = pb.tile([128, 512], f32, tag="rden",
                                           bufs=2, name=f"rden_{h}_{g}")
                            nc.gpsimd.partition_broadcast(rden[:], rd1[:])
                            nc.vector.tensor_mul(
                                attn[:, g * 512:(g + 1) * 512], po[:],
                                rden[:])
                        nc.sync.dma_start(out=attn_d[h][:],
                                          in_=attn[:].bitcast(f32))
                        if h < HPC - 1:
                            nc.gpsimd.collective_compute(
                                "AllGather", mybir.AluOpType.bypass,
                                replica_groups=GROUPS,
                                ins=[attn_d[h][:]], outs=[attn_ag[:][h]])

            # last head's AllGather sits outside the stage-B pool scope so
            # the pool release (and stage C's start) does not wait for it
            nc.gpsimd.collective_compute(
                "AllGather", mybir.AluOpType.bypass, replica_groups=GROUPS,
                ins=[attn_d[HPC - 1][:]], outs=[attn_ag7[:]])

            # -------- stage C: o_proj, heads 0-6 main + head-7 increment -----
            with nc.named_scope("stageC"), \
                 tc.tile_pool(name="stC", bufs=1) as pc, \
                 tc.tile_pool(name="psC", bufs=4, space="PSUM") as psC:
                wo_h1 = pc.tile([128, NIB, JC // 2], f32r, tag="wo1", bufs=1)
                for d in range(8):
                    nc.sync.dma_start(
                        out=wo_h1[:, 4 * d:4 * (d + 1), :],
                        in_=woT_v[:, 4 * d:4 * (d + 1), JC // 2:].bitcast(f32r))
                for tb in range(NKB):
                    at_sb = pc.tile([128, NJB_MAIN, 128], f32r, tag="atC",
                                    bufs=3, name=f"atC_{tb}")
                    nc.sync.dma_start(
                        out=at_sb[:],
                        in_=ag_v[:, :, tb * 128:(tb + 1) * 128].bitcast(f32r))
                    a7_sb = pc.tile([128, TPN, 128], f32r, tag="a7C",
                                    bufs=3, name=f"a7C_{tb}")
                    nc.sync.dma_start(
                        out=a7_sb[:],
                        in_=ag7_v[:, :, tb * 128:(tb + 1) * 128].bitcast(f32r))
                    for mc in range(JC // 512):
                        wsrc = wo_h0 if mc == 0 else wo_h1
                        psc = psC.tile([128, 512], f32, tag="psC",
                                       name=f"psC_{tb}_{mc}")
                        for jb in range(NJB_MAIN):
                            nc.tensor.matmul(
                                psc[:], at_sb[:, jb, :], wsrc[:, jb, :],
                                start=(jb == 0), stop=(jb == NJB_MAIN - 1))
                        main_sb = pc.tile([128, 512], f32, tag="mainC",
                                          bufs=4, name=f"mainC_{tb}_{mc}")
                        nc.scalar.copy(main_sb[:], psc[:])
                        ps7 = psC.tile([128, 512], f32, tag="ps7", bufs=2,
                                       name=f"ps7_{tb}_{mc}")
                        for i in range(TPN):
                            nc.tensor.matmul(
                                ps7[:], a7_sb[:, i, :],
                                wsrc[:, NJB_MAIN + i, :],
                                start=(i == 0), stop=(i == TPN - 1))
                        oc = pc.tile([128, 512], f32, tag="oC", bufs=4,
                                     name=f"oC_{tb}_{mc}")
                        nc.vector.tensor_add(oc[:], ps7[:], main_sb[:])
                        nc.sync.dma_start(
                            out=out[:][tb * 128:(tb + 1) * 128,
                                       mc * 512:(mc + 1) * 512],
                            in_=oc[:])

    nc.finalize()
    return nc


_NC_CACHE = None


def _get_nc():
    global _NC_CACHE
    if _NC_CACHE is None:
        _NC_CACHE = build_nc()
    return _NC_CACHE


def _host_inputs(hidden_states, positions, w_pack, w_o):
    hidden_states = np.asarray(hidden_states, dtype=np.float32)
    positions = np.asarray(positions)
    w_pack = np.asarray(w_pack, dtype=np.float32)
    w_o = np.asarray(w_o, dtype=np.float32)

    half = HD // 2
    inv_freq = (1.0 / (THETA ** (np.arange(half, dtype=np.float32) / half)))

    # causal mask variants for the 4 diagonal (128x512) tiles of a q-block
    masks = np.empty((4, 128, 512), dtype=np.float32)
    xs = np.arange(512)[None, :]
    ps = np.arange(128)[:, None]
    for v in range(4):
        masks[v] = (xs >= ps + 128 * v).astype(np.float32)

    in_maps = []
    for c in range(NCORES):
        b, r = divmod(c, TPN)
        heads = np.arange(HPC * r, HPC * (r + 1))
        rows = (heads[:, None] * HD + np.arange(HD)[None, :]).reshape(-1)
        w_core = np.concatenate(
            [w_pack[rows], w_pack[H + rows], w_pack[2 * H + rows]], axis=0)
        wT = np.ascontiguousarray(w_core.T)                      # [H, 3*JC]
        # o_proj m-shard rows, j-order permuted to match AllGather layout:
        # gathered row (h, r', d) holds global head 8*r'+h
        wo_shard = w_o[JC * r:JC * (r + 1), :]                   # [JC, H]
        woT_full = np.ascontiguousarray(wo_shard.T)              # [H=j, JC]
        woT_perm = woT_full.reshape(TPN, HPC, HD, JC) \
                           .transpose(1, 0, 2, 3).reshape(H, JC)
        hsT = np.ascontiguousarray(hidden_states[b].T)           # [H, S]
        ang = positions[b].astype(np.float32)[None, :] * inv_freq[:, None]
        cos_t = np.cos(ang).astype(np.float32)                   # [64, S]
        sin_t = np.sin(ang).astype(np.float32)
        cosf = np.concatenate([cos_t, cos_t], axis=0)            # [128, S]
        sinm = np.concatenate([-sin_t, sin_t], axis=0)
        in_maps.append({
            "hsT": hsT, "wT": wT, "woT": np.ascontiguousarray(woT_perm),
            "cosf": cosf.astype(ml_dtypes.bfloat16),
            "sinm": sinm.astype(ml_dtypes.bfloat16),
            "masks": masks.astype(ml_dtypes.bfloat16),
        })
    return in_maps


def kernel(hidden_states, positions, w_pack, w_o):
    import os
    os.environ["BASS_NEVER_TRACE"] = "1"
    nc = _get_nc()
    in_maps = _host_inputs(hidden_states, positions, w_pack, w_o)
    res = run_bass_kernel_spmd(nc, in_maps, list(range(NCORES)))
    out = np.empty((B, S, H), dtype=np.float32)
    for c in range(NCORES):
        b, r = divmod(c, TPN)
        out[b][:, JC * r:JC * (r + 1)] = res.results[c]["out"]
    return out



# revision 5
# speedup vs baseline: 1.0388x; 1.0388x over previous
"""BaiChuan attention layer on 8 Trainium2 NeuronCores.

Sharding: tensor-parallel over heads within groups of 4 cores (W_pack
column-parallel, o_proj column-parallel after a per-head AllGather of
attention outputs), data-parallel over the batch across the two groups.

v2: all matmul operands in bf16 (halves DMA + SBUF vs fp32, same PE
streaming rate), stage A split into a q/k pass (w stationary, hs
moving) and a v pass producing v in natural [token, hd] layout (hs
stationary, wv moving) so stage B needs no PE transposes; softmax
denominator moved off the PE (DVE accumulate + GpSimd partition
reduce); exp batched 1024-wide on ACT; AllGathers in bf16.

Per-core dataflow (core c: batch b=c//4, rank r=c%4, heads 8r..8r+8):
  stage A-qk: for th token-block: qkT[jt] = W_jt @ hs_th   (16 jts: k0,q0,..)
  stage A-v:  for tb 128-token block: v[tb] = hs_tb.T @ Wv  (SBUF resident)
  stage B: per head: neox RoPE on qT,kT (DVE), causal attention with
           sT = kT-blocks.T @ qT, batched exp on ACT, denominator via
           DVE accumulate + gpsimd partition-reduce, PV from resident v,
           per-head AllGather (bf16) overlapping later heads' compute.
  stage C: o_proj column-parallel over the gathered head dim, split as
           heads 0-6 + head-7 increment.  Host concatenates m-shards.
"""
import sys
sys.path.insert(0, '/opt/trn_rl_repo')
import numpy as np
import ml_dtypes

import concourse.bass as bass
from concourse import bacc
import concourse.mybir as mybir
from concourse.tile import TileContext
from concourse.bass_utils import run_bass_kernel_spmd
from concourse import bass_isa

f32 = mybir.dt.float32
bf16 = mybir.dt.bfloat16
AF = mybir.ActivationFunctionType

B, S, H, NH = 2, 2048, 4096, 32
HD = H // NH                    # 128
THETA = 10000.0
NCORES, TPN = 8, 4              # 2 groups of 4 (DP over batch x TP over heads)
HPC = NH // TPN                 # 8 heads per core
JC = HPC * HD                   # 1024 per-core q (=k=v) width
SCALE = HD ** -0.5
GROUPS = [[0, 1, 2, 3], [4, 5, 6, 7]]
NIB = H // 128                  # 32 contraction blocks
NJT = 2 * HPC                   # 16 q/k row-tiles in stage A (k0,q0,k1,q1..)
NG = S // 512                   # 4 query blocks per head
NKB = S // 128                  # 16 key blocks per head
NTB = NKB                       # 16 token 128-blocks
NJB = TPN * HPC                 # 32 o_proj contraction blocks
NJB_MAIN = NJB - TPN            # heads 0..6 -> jb 0..27


def build_nc():
    nc = bacc.Bacc(None)
    hsT = nc.declare_dram_parameter("hsT", [H, S], bf16, isOutput=False)
    wqkT = nc.declare_dram_parameter("wqkT", [H, NJT * 128], bf16,
                                     isOutput=False)
    wvT = nc.declare_dram_parameter("wvT", [H, JC], bf16, isOutput=False)
    woT = nc.declare_dram_parameter("woT", [H, JC], bf16, isOutput=False)
    cosf = nc.declare_dram_parameter("cosf", [HD, S], bf16, isOutput=False)
    sinm = nc.declare_dram_parameter("sinm", [HD, S], bf16, isOutput=False)
    masks = nc.declare_dram_parameter("masks", [4, 128, 512], bf16,
                                      isOutput=False)
    out = nc.declare_dram_parameter("out", [S, JC], f32, isOutput=True)

    qk_d = [nc.dram_tensor(f"qk_d{j}", [128, S], bf16) for j in range(NJT)]
    attn_d = [nc.dram_tensor(f"attn_d{h}", [HD, S], bf16) for h in range(HPC)]
    attn_ag = nc.dram_tensor("attn_ag", [HPC - 1, TPN * HD, S], bf16)
    attn_ag7 = nc.dram_tensor("attn_ag7", [TPN * HD, S], bf16)

    hsT_v = hsT[:].rearrange("(n p) t -> p n t", p=128)      # [128, 32, S]
    wqkT_v = wqkT[:].rearrange("(n p) j -> p n j", p=128)    # [128, 32, 2048]
    wvT_v = wvT[:].rearrange("(n p) j -> p n j", p=128)      # [128, 32, JC]
    woT_v = woT[:].rearrange("(n p) m -> p n m", p=128)      # [128, 32, JC]
    ag_v = attn_ag[:].rearrange("h (r p) t -> p (h r) t", p=128)  # [128,28,S]
    ag7_v = attn_ag7[:].rearrange("(r p) t -> p r t", p=128)       # [128,4,S]

    with TileContext(nc) as tc:
        # persistent pool: resident v in natural [token, hd] layout
        with tc.tile_pool(name="P0", bufs=1) as p0:
            v_sb = p0.tile([128, NTB, JC], bf16, tag="v_sb", bufs=1)

            # wv weight pool spans both stage-A passes only
            with tc.tile_pool(name="PAW", bufs=1) as paw:
                wv_sb = paw.tile([128, NIB, JC], bf16, tag="wv", bufs=1)
                for d in range(8):
                    nc.sync.dma_start(out=wv_sb[:, 4 * d:4 * (d + 1), :],
                                      in_=wvT_v[:, 4 * d:4 * (d + 1), :])

                # ---------------- stage A-qk: q/k projection ----------------
                with nc.named_scope("stageAqk"), \
                     tc.tile_pool(name="stA", bufs=1) as pa, \
                     tc.tile_pool(name="psA", bufs=6, space="PSUM") as psA:
                    for th in range(S // 512):
                        hs_th = pa.tile([128, NIB, 512], bf16, tag="hs",
                                        bufs=2, name=f"hs_{th}")
                        for d in range(8):
                            nc.sync.dma_start(
                                out=hs_th[:, 4 * d:4 * (d + 1), :],
                                in_=hsT_v[:, 4 * d:4 * (d + 1),
                                          th * 512:(th + 1) * 512])
                        for jt in range(NJT):
                            w_sb = pa.tile([128, NIB, 128], bf16, tag="w",
                                           bufs=4, name=f"w_{th}_{jt}")
                            nc.sync.dma_start(
                                out=w_sb[:],
                                in_=wqkT_v[:, :, jt * 128:(jt + 1) * 128])
                            ps = psA.tile([128, 512], f32, tag="psA",
                                          name=f"psA_{th}_{jt}")
                            for ib in range(NIB):
                                nc.tensor.matmul(
                                    ps[:], w_sb[:, ib, :], hs_th[:, ib, :],
                                    start=(ib == 0), stop=(ib == NIB - 1))
                            st = pa.tile([128, 512], bf16, tag="oA", bufs=4,
                                         name=f"stA_{th}_{jt}")
                            nc.scalar.copy(st[:], ps[:])
                            nc.sync.dma_start(
                                out=qk_d[jt][:][:, th * 512:(th + 1) * 512],
                                in_=st[:])

                # ------------- stage A-v: v in natural layout ---------------
                with nc.named_scope("stageAv"), \
                     tc.tile_pool(name="stV", bufs=1) as pv, \
                     tc.tile_pool(name="psV", bufs=6, space="PSUM") as psV:
                    for tb in range(NTB):
                        hs_tb = pv.tile([128, NIB, 128], bf16, tag="hsv",
                                        bufs=3, name=f"hsv_{tb}")
                        for d in range(2):
                            nc.sync.dma_start(
                                out=hs_tb[:, 16 * d:16 * (d + 1), :],
                                in_=hsT_v[:, 16 * d:16 * (d + 1),
                                          tb * 128:(tb + 1) * 128])
                        for mc in range(JC // 512):
                            ps = psV.tile([128, 512], f32, tag="psV",
                                          name=f"psV_{tb}_{mc}")
                            for ib in range(NIB):
                                nc.tensor.matmul(
                                    ps[:], hs_tb[:, ib, :],
                                    wv_sb[:, ib, mc * 512:(mc + 1) * 512],
                                    start=(ib == 0), stop=(ib == NIB - 1))
                            nc.vector.tensor_copy(
                                v_sb[:, tb, mc * 512:(mc + 1) * 512], ps[:])

            # ------------- stages B+C share the o_proj weight pool -------
            with tc.tile_pool(name="stWo", bufs=1, side="right") as pwo:
                wo_h0 = pwo.tile([128, NIB, JC // 2], bf16, tag="wo0", bufs=1)

                # ------------- stage B: rope + causal attention ----------
                with nc.named_scope("stageB"), \
                     tc.tile_pool(name="stB", bufs=1) as pb, \
                     tc.tile_pool(name="psB", bufs=1, space="PSUM") as psB:
                    cos_sb = pb.tile([128, S], bf16, tag="cos", bufs=1)
                    sin_sb = pb.tile([128, S], bf16, tag="sin", bufs=1)
                    nc.sync.dma_start(out=cos_sb[:], in_=cosf[:])
                    nc.sync.dma_start(out=sin_sb[:], in_=sinm[:])
                    mask_sb = pb.tile([128, 4, 512], bf16, tag="mask", bufs=1)
                    nc.sync.dma_start(out=mask_sb[:],
                                      in_=masks[:].rearrange("v p x -> p v x"))
                    for d in range(8):
                        nc.sync.dma_start(
                            out=wo_h0[:, 4 * d:4 * (d + 1), :],
                            in_=woT_v[:, 4 * d:4 * (d + 1), :JC // 2])

                    def load_rope(jt, tag, h):
                        """load qk_d row-block jt, apply neox rope (bf16)"""
                        raw = pb.tile([128, S], bf16, tag="raw", bufs=6,
                                      name=f"{tag}_raw_{h}")
                        nc.sync.dma_start(out=raw[:], in_=qk_d[jt][:])
                        sw = pb.tile([128, S], bf16, tag="raw", bufs=6,
                                     name=f"{tag}_sw_{h}")
                        nc.sync.dma_start(out=sw[0:64, :],
                                          in_=qk_d[jt][:][64:128, :])
                        nc.sync.dma_start(out=sw[64:128, :],
                                          in_=qk_d[jt][:][0:64, :])
                        t1 = pb.tile([128, S], bf16, tag="ropetmp", bufs=2,
                                     name=f"{tag}_t1_{h}")
                        t2 = pb.tile([128, S], bf16, tag="ropetmp", bufs=2,
                                     name=f"{tag}_t2_{h}")
                        with tc.high_priority():
                            nc.vector.tensor_mul(t1[:], raw[:], cos_sb[:])
                            nc.vector.tensor_mul(t2[:], sw[:], sin_sb[:])
                            rt = pb.tile([128, S], bf16, tag=f"{tag}_r",
                                         bufs=2, name=f"{tag}_roped_{h}")
                            nc.vector.tensor_add(rt[:], t1[:], t2[:])
                        return rt

                    for h in range(HPC):
                        with nc.named_scope(f"head{h}"):
                            kT = load_rope(2 * h, "kr", h)
                            qT = load_rope(2 * h + 1, "qr", h)
                            hc0 = h * 128
                            attn = pb.tile([128, S], bf16, tag="attn", bufs=2,
                                           name=f"attn_{h}")
                            for g in range(NG):
                                nu = 2 * g + 2      # 1024-wide units
                                po = psB.tile([128, 512], f32, tag="po",
                                              bufs=2, name=f"po_{h}_{g}")
                                acc2 = pb.tile([128, 1024], f32, tag="acc2",
                                               bufs=2, name=f"acc2_{h}_{g}")
                                for u in range(nu):
                                    ps2 = psB.tile([128, 1024], f32,
                                                   tag="pss", bufs=3,
                                                   name=f"pss_{h}_{g}_{u}")
                                    for half in range(2):
                                        kb = 2 * u + half
                                        nc.tensor.matmul(
                                            ps2[:, half * 512:
                                                (half + 1) * 512],
                                            kT[:, kb * 128:(kb + 1) * 128],
                                            qT[:, g * 512:(g + 1) * 512],
                                            start=True, stop=True)
                                    pt = pb.tile([128, 1024], bf16, tag="pt",
                                                 bufs=3,
                                                 name=f"pt_{h}_{g}_{u}")
                                    nc.scalar.activation(pt[:], ps2[:],
                                                         AF.Exp, scale=SCALE)
                                    if u >= 2 * g:
                                        mj = 2 * (u - 2 * g)
                                        nc.vector.tensor_mul(
                                            pt[:].rearrange(
                                                "p (v x) -> p v x", v=2),
                                            pt[:].rearrange(
                                                "p (v x) -> p v x", v=2),
                                            mask_sb[:, mj:mj + 2, :])
                                    if u == 0:
                                        nc.vector.tensor_copy(acc2[:], pt[:])
                                    else:
                                        nc.vector.tensor_add(acc2[:],
                                                             acc2[:], pt[:])
                                    for half in range(2):
                                        kb = 2 * u + half
                                        nc.tensor.matmul(
                                            po[:],
                                            v_sb[:, kb, hc0:hc0 + 128],
                                            pt[:, half * 512:
                                               (half + 1) * 512],
                                            start=(kb == 0),
                                            stop=(kb == 2 * nu - 1))
                                den = pb.tile([128, 512], f32, tag="den",
                                              bufs=2, name=f"den_{h}_{g}")
                                nc.vector.tensor_add(den[:], acc2[:, 0:512],
                                                     acc2[:, 512:1024])
                                denall = pb.tile([128, 512], f32,
                                                 tag="denall", bufs=2,
                                                 name=f"denall_{h}_{g}")
                                nc.gpsimd.partition_all_reduce(
                                    denall[:], den[:], 128,
                                    bass_isa.ReduceOp.add)
                                rden = pb.tile([128, 512], f32, tag="rden",
                                               bufs=2, name=f"rden_{h}_{g}")
                                nc.vector.reciprocal(rden[:], denall[:])
                                nc.vector.tensor_mul(
                                    attn[:, g * 512:(g + 1) * 512], po[:],
                                    rden[:])
                            nc.sync.dma_start(out=attn_d[h][:], in_=attn[:])
                            if h < HPC - 1:
                                nc.gpsimd.collective_compute(
                                    "AllGather", mybir.AluOpType.bypass,
                                    replica_groups=GROUPS,
                                    ins=[attn_d[h][:]], outs=[attn_ag[:][h]])

                # last head's AllGather sits outside the stage-B pool scope
                nc.gpsimd.collective_compute(
                    "AllGather", mybir.AluOpType.bypass, replica_groups=GROUPS,
                    ins=[attn_d[HPC - 1][:]], outs=[attn_ag7[:]])

                # ------ stage C: o_proj, heads 0-6 main + head-7 increment ---
                with nc.named_scope("stageC"), \
                     tc.tile_pool(name="stC", bufs=1) as pc, \
                     tc.tile_pool(name="psC", bufs=4, space="PSUM") as psC:
                    wo_h1 = pc.tile([128, NIB, JC // 2], bf16, tag="wo1",
                                    bufs=1)
                    for d in range(8):
                        nc.sync.dma_start(
                            out=wo_h1[:, 4 * d:4 * (d + 1), :],
                            in_=woT_v[:, 4 * d:4 * (d + 1), JC // 2:])
                    for tb in range(NKB):
                        at_sb = pc.tile([128, NJB_MAIN, 128], bf16, tag="atC",
                                        bufs=3, name=f"atC_{tb}")
                        nc.sync.dma_start(
                            out=at_sb[:],
                            in_=ag_v[:, :, tb * 128:(tb + 1) * 128])
                        a7_sb = pc.tile([128, TPN, 128], bf16, tag="a7C",
                                        bufs=3, name=f"a7C_{tb}")
                        nc.sync.dma_start(
                            out=a7_sb[:],
                            in_=ag7_v[:, :, tb * 128:(tb + 1) * 128])
                        for mc in range(JC // 512):
                            wsrc = wo_h0 if mc == 0 else wo_h1
                            psc = psC.tile([128, 512], f32, tag="psC",
                                           name=f"psC_{tb}_{mc}")
                            for jb in range(NJB_MAIN):
                                nc.tensor.matmul(
                                    psc[:], at_sb[:, jb, :], wsrc[:, jb, :],
                                    start=(jb == 0), stop=(jb == NJB_MAIN - 1))
                            main_sb = pc.tile([128, 512], f32, tag="mainC",
                                              bufs=4, name=f"mainC_{tb}_{mc}")
                            nc.scalar.copy(main_sb[:], psc[:])
                            ps7 = psC.tile([128, 512], f32, tag="ps7", bufs=2,
                                           name=f"ps7_{tb}_{mc}")
                            for i in range(TPN):
                                nc.tensor.matmul(
                                    ps7[:], a7_sb[:, i, :],
                                    wsrc[:, NJB_MAIN + i, :],
                                    start=(i == 0), stop=(i == TPN - 1))
                            oc = pc.tile([128, 512], f32, tag="oC", bufs=4,
                                         name=f"oC_{tb}_{mc}")
                            nc.vector.tensor_add(oc[:], ps7[:], main_sb[:])
                            nc.sync.dma_start(
                                out=out[:][tb * 128:(tb + 1) * 128,
                                           mc * 512:(mc + 1) * 512],
                                in_=oc[:])

    nc.finalize()
    return nc


_NC_CACHE = None


def _get_nc():
    global _NC_CACHE
    if _NC_CACHE is None:
        _NC_CACHE = build_nc()
    return _NC_CACHE


def _host_inputs(hidden_states, positions, w_pack, w_o):
    hidden_states = np.asarray(hidden_states, dtype=np.float32)
    positions = np.asarray(positions)
    w_pack = np.asarray(w_pack, dtype=np.float32)
    w_o = np.asarray(w_o, dtype=np.float32)

    half = HD // 2
    inv_freq = (1.0 / (THETA ** (np.arange(half, dtype=np.float32) / half)))

    # causal mask variants for the 4 diagonal (128x512) tiles of a q-block
    masks = np.empty((4, 128, 512), dtype=np.float32)
    xs = np.arange(512)[None, :]
    ps = np.arange(128)[:, None]
    for v in range(4):
        masks[v] = (xs >= ps + 128 * v).astype(np.float32)

    in_maps = []
    for c in range(NCORES):
        b, r = divmod(c, TPN)
        heads = np.arange(HPC * r, HPC * (r + 1))
        rows = (heads[:, None] * HD + np.arange(HD)[None, :]).reshape(-1)
        Wq = w_pack[rows]                                        # [JC, H]
        Wk = w_pack[H + rows]
        Wv = w_pack[2 * H + rows]
        # qk weight columns interleaved per head: k_h then q_h
        wqk = np.empty((NJT * 128, H), dtype=np.float32)
        for h in range(HPC):
            wqk[256 * h:256 * h + 128] = Wk[128 * h:128 * (h + 1)]
            wqk[256 * h + 128:256 * (h + 1)] = Wq[128 * h:128 * (h + 1)]
        wqkT = np.ascontiguousarray(wqk.T)                       # [H, 2048]
        wvT = np.ascontiguousarray(Wv.T)                         # [H, JC]
        # o_proj m-shard rows, j-order permuted to match AllGather layout:
        # gathered row (h, r', d) holds global head 8*r'+h
        wo_shard = w_o[JC * r:JC * (r + 1), :]                   # [JC, H]
        woT_full = np.ascontiguousarray(wo_shard.T)              # [H=j, JC]
        woT_perm = woT_full.reshape(TPN, HPC, HD, JC) \
                           .transpose(1, 0, 2, 3).reshape(H, JC)
        hsT = np.ascontiguousarray(hidden_states[b].T)           # [H, S]
        ang = positions[b].astype(np.float32)[None, :] * inv_freq[:, None]
        cos_t = np.cos(ang).astype(np.float32)                   # [64, S]
        sin_t = np.sin(ang).astype(np.float32)
        cosf = np.concatenate([cos_t, cos_t], axis=0)            # [128, S]
        sinm = np.concatenate([-sin_t, sin_t], axis=0)
        in_maps.append({
            "hsT": hsT.astype(ml_dtypes.bfloat16),
            "wqkT": wqkT.astype(ml_dtypes.bfloat16),
            "wvT": wvT.astype(ml_dtypes.bfloat16),
            "woT": np.ascontiguousarray(woT_perm).astype(ml_dtypes.bfloat16),
            "cosf": cosf.astype(ml_dtypes.bfloat16),
            "sinm": sinm.astype(ml_dtypes.bfloat16),
            "masks": masks.astype(ml_dtypes.bfloat16),
        })
    return in_maps


def kernel(hidden_states, positions, w_pack, w_o):
    import os
    os.environ["BASS_NEVER_TRACE"] = "1"
    nc = _get_nc()
    in_maps = _host_inputs(hidden_states, positions, w_pack, w_o)
    res = run_bass_kernel_spmd(nc, in_maps, list(range(NCORES)))
    out = np.empty((B, S, H), dtype=np.float32)
    for c in range(NCORES):
        b, r = divmod(c, TPN)
        out[b][:, JC * r:JC * (r + 1)] = res.results[c]["out"]
    return out


# revision 6
# speedup vs baseline: 1.2264x; 1.1806x over previous
"""BaiChuan attention layer on 8 Trainium2 NeuronCores.

Sharding: tensor-parallel over heads within groups of 4 cores (W_pack
column-parallel, o_proj column-parallel after per-head-pair AllGathers
of attention outputs), data-parallel over the batch across the groups.

v3: q/k projection in fp8-e4m3 with DoubleRow matmuls (K=256 per MM,
inputs pre-scaled x64 on host, rescaled 2^-12 on the PSUM-evacuate
copy); v projected in bf16 directly into natural [token, hd] layout
(SBUF-resident, no PE transposes); causal mask applied as a -1e4
PSUM-init via identity matmul (off the DVE); softmax denominator as
accumulated [1,512] ones-matmuls on the PE + reciprocal_approx_fast;
AllGathers batched per head-pair (4 ops) in bf16; o_proj split as
heads 0-5 main + heads 6-7 increment.

Per-core dataflow (core c: batch b=c//4, rank r=c%4, heads 8r..8r+8):
  stage A-qk: for th token-block: qkT[jt] = Wqk_jt @ hs_th   (fp8 DR)
  stage A-v:  for tb 128-token block: v[tb] = hs_tb.T @ Wv   (bf16)
  stage B: per head: neox RoPE on qT,kT (DVE), causal attention with
           sT = kT-blocks.T @ qT, batched exp on ACT, PV from resident
           v, pair AllGathers overlapping later heads' compute.
  stage C: o_proj column-parallel over the gathered head dim.
"""
import sys
sys.path.insert(0, '/opt/trn_rl_repo')
import numpy as np
import ml_dtypes

import concourse.bass as bass
from concourse import bacc
import concourse.mybir as mybir
from concourse.tile import TileContext
from concourse.bass_utils import run_bass_kernel_spmd
from concourse.masks import make_identity

f32 = mybir.dt.float32
bf16 = mybir.dt.bfloat16
fp8 = mybir.dt.float8e4
AF = mybir.ActivationFunctionType
DR = mybir.MatmulPerfMode.DoubleRow

B, S, H, NH = 2, 2048, 4096, 32
HD = H // NH                    # 128
THETA = 10000.0
NCORES, TPN = 8, 4              # 2 groups of 4 (DP over batch x TP over heads)
HPC = NH // TPN                 # 8 heads per core
JC = HPC * HD                   # 1024 per-core q (=k=v) width
SCALE = HD ** -0.5
FP8_S = 64.0                    # host pre-scale on hs8/wqk8
UNSCALE = 1.0 / (FP8_S * FP8_S)  # 2^-12 rescale on the qk PSUM evacuate
GROUPS = [[0, 1, 2, 3], [4, 5, 6, 7]]
NIB = H // 128                  # 32 contraction blocks
NJT = 2 * HPC                   # 16 q/k row-tiles in stage A (k0,q0,k1,q1..)
NG = S // 512                   # 4 query blocks per head
NKB = S // 128                  # 16 key blocks per head
NTB = NKB                       # 16 token 128-blocks
NPAIR = HPC // 2                # 4 AllGather head-pairs
NJB_MAIN = 3 * TPN * 2          # pairs 0-2 -> 24 o_proj jb blocks
NJB_TAIL = TPN * 2              # pair 3 -> 8 jb blocks


def build_nc():
    nc = bacc.Bacc(None)
    hsT = nc.declare_dram_parameter("hsT", [H, S], bf16, isOutput=False)
    hs8 = nc.declare_dram_parameter("hs8", [H, S], fp8, isOutput=False)
    wqk8 = nc.declare_dram_parameter("wqk8", [H, NJT * 128], fp8,
                                     isOutput=False)
    wvT = nc.declare_dram_parameter("wvT", [H, JC], bf16, isOutput=False)
    woT = nc.declare_dram_parameter("woT", [H, JC], bf16, isOutput=False)
    cosf = nc.declare_dram_parameter("cosf", [HD, S], bf16, isOutput=False)
    sinm = nc.declare_dram_parameter("sinm", [HD, S], bf16, isOutput=False)
    lmask = nc.declare_dram_parameter("lmask", [4, 128, 512], bf16,
                                      isOutput=False)
    out = nc.declare_dram_parameter("out", [S, JC], f32, isOutput=True)

    qk_d = [nc.dram_tensor(f"qk_d{j}", [128, S], bf16) for j in range(NJT)]
    attn_d = nc.dram_tensor("attn_d", [HPC, HD, S], bf16)
    attn_ag = nc.dram_tensor("attn_ag", [NPAIR, TPN * 2 * HD, S], bf16)

    hsT_v = hsT[:].rearrange("(n p) t -> p n t", p=128)      # [128, 32, S]
    hs8_v = hs8[:].rearrange("(n p) t -> p n t", p=128)      # [128, 32, S]
    wqk8_v = wqk8[:].rearrange("(n p) j -> p n j", p=128)    # [128, 32, 2048]
    wvT_v = wvT[:].rearrange("(n p) j -> p n j", p=128)      # [128, 32, JC]
    woT_v = woT[:].rearrange("(n p) m -> p n m", p=128)      # [128, 32, JC]
    ag_main = attn_ag[:][0:3].rearrange("p (x q) t -> q (p x) t", q=128)
    ag_tail = attn_ag[:][3].rearrange("(x q) t -> q x t", q=128)

    with TileContext(nc) as tc:
        # persistent pool: resident v (natural layout) + stage-B prep tiles
        with tc.tile_pool(name="P0", bufs=1) as p0:
            v_sb = p0.tile([128, NTB, JC], bf16, tag="v_sb", bufs=1)

            # wv weight pool spans both stage-A passes only
            with tc.tile_pool(name="PAW", bufs=1) as paw:
                wv_sb = paw.tile([128, NIB, JC], bf16, tag="wv", bufs=1)

                # ------------- stage A-qk: q/k projection (fp8 DR) ----------
                with nc.named_scope("stageAqk"), \
                     tc.tile_pool(name="stA", bufs=1) as pa, \
                     tc.tile_pool(name="psA", bufs=6, space="PSUM") as psA:
                    hs_tiles = {}

                    def load_hs8(th):
                        t = pa.tile([128, NIB, 512], fp8, tag="hs", bufs=2,
                                    name=f"hs_{th}")
                        for d in range(4):
                            nc.sync.dma_start(
                                out=t[:, 8 * d:8 * (d + 1), :],
                                in_=hs8_v[:, 8 * d:8 * (d + 1),
                                          th * 512:(th + 1) * 512])
                        hs_tiles[th] = t

                    load_hs8(0)
                    for th in range(S // 512):
                        if th + 1 < S // 512:
                            load_hs8(th + 1)
                        # wv chunks spread across th iterations (2 per th)
                        for d in (2 * th, 2 * th + 1):
                            nc.sync.dma_start(
                                out=wv_sb[:, 4 * d:4 * (d + 1), :],
                                in_=wvT_v[:, 4 * d:4 * (d + 1), :])
                        hs_th = hs_tiles.pop(th)
                        for jt in range(NJT):
                            w_sb = pa.tile([128, NIB, 128], fp8, tag="w",
                                           bufs=4, name=f"w_{th}_{jt}")
                            nc.sync.dma_start(
                                out=w_sb[:],
                                in_=wqk8_v[:, :, jt * 128:(jt + 1) * 128])
                            ps = psA.tile([128, 512], f32, tag="psA",
                                          name=f"psA_{th}_{jt}")
                            for i2 in range(NIB // 2):
                                nc.tensor.matmul(
                                    ps[:], w_sb[:, 2 * i2:2 * i2 + 2, :],
                                    hs_th[:, 2 * i2:2 * i2 + 2, :],
                                    start=(i2 == 0),
                                    stop=(i2 == NIB // 2 - 1),
                                    perf_mode=DR)
                            st = pa.tile([128, 512], bf16, tag="oA", bufs=4,
                                         name=f"stA_{th}_{jt}")
                            nc.scalar.mul(st[:], ps[:], UNSCALE)
                            nc.sync.dma_start(
                                out=qk_d[jt][:][:, th * 512:(th + 1) * 512],
                                in_=st[:])

                # stage-B prep at P0 level: loads overlap stage A-v
                cos_sb = p0.tile([128, S], bf16, tag="cos", bufs=1)
                sin_sb = p0.tile([128, S], bf16, tag="sin", bufs=1)
                lm_sb = p0.tile([128, 4, 512], bf16, tag="lmask", bufs=1)
                ident = p0.tile([128, 128], bf16, tag="ident", bufs=1)
                ones_b = p0.tile([128, 1], bf16, tag="ones", bufs=1)

                # ------------- stage A-v: v in natural layout (bf16) --------
                with nc.named_scope("stageAv"), \
                     tc.tile_pool(name="stV", bufs=1) as pv, \
                     tc.tile_pool(name="psV", bufs=6, space="PSUM") as psV:
                    hsv_tiles = {}

                    def load_hsv(tb):
                        t = pv.tile([128, NIB, 128], bf16, tag="hsv", bufs=3,
                                    name=f"hsv_{tb}")
                        for d in range(2):
                            nc.sync.dma_start(
                                out=t[:, 16 * d:16 * (d + 1), :],
                                in_=hsT_v[:, 16 * d:16 * (d + 1),
                                          tb * 128:(tb + 1) * 128])
                        hsv_tiles[tb] = t

                    load_hsv(0)
                    load_hsv(1)
                    # prep-tile fills run during stage A-v
                    nc.sync.dma_start(out=cos_sb[:], in_=cosf[:])
                    nc.sync.dma_start(out=sin_sb[:], in_=sinm[:])
                    nc.sync.dma_start(out=lm_sb[:],
                                      in_=lmask[:].rearrange("v p x -> p v x"))
                    make_identity(nc, ident[:])
                    nc.vector.memset(ones_b[:], 1.0)
                    for tb in range(NTB):
                        if tb + 2 < NTB:
                            load_hsv(tb + 2)
                        hs_tb = hsv_tiles.pop(tb)
                        for mc in range(JC // 512):
                            ps = psV.tile([128, 512], f32, tag="psV",
                                          name=f"psV_{tb}_{mc}")
                            for ib in range(NIB):
                                nc.tensor.matmul(
                                    ps[:], hs_tb[:, ib, :],
                                    wv_sb[:, ib, mc * 512:(mc + 1) * 512],
                                    start=(ib == 0), stop=(ib == NIB - 1))
                            nc.vector.tensor_copy(
                                v_sb[:, tb, mc * 512:(mc + 1) * 512], ps[:])

            # ------------- stages B+C share the o_proj weight pool ----------
            with tc.tile_pool(name="stWo", bufs=1, side="right") as pwo:
                wo_h0 = pwo.tile([128, NIB, JC // 2], bf16, tag="wo0", bufs=1)

                # ------------- stage B: rope + causal attention -------------
                with nc.named_scope("stageB"), \
                     tc.tile_pool(name="stB", bufs=1) as pb, \
                     tc.tile_pool(name="psB", bufs=1, space="PSUM") as psB:

                    def load_rope(jt, tag, h):
                        """load qk_d row-block jt, apply neox rope (bf16)"""
                        raw = pb.tile([128, S], bf16, tag="raw", bufs=8,
                                      name=f"{tag}_raw_{h}")
                        nc.sync.dma_start(out=raw[:], in_=qk_d[jt][:])
                        sw = pb.tile([128, S], bf16, tag="raw", bufs=8,
                                     name=f"{tag}_sw_{h}")
                        nc.sync.dma_start(out=sw[0:64, :],
                                          in_=qk_d[jt][:][64:128, :])
                        nc.sync.dma_start(out=sw[64:128, :],
                                          in_=qk_d[jt][:][0:64, :])
                        t1 = pb.tile([128, S], bf16, tag="ropetmp", bufs=2,
                                     name=f"{tag}_t1_{h}")
                        t2 = pb.tile([128, S], bf16, tag="ropetmp", bufs=2,
                                     name=f"{tag}_t2_{h}")
                        with tc.high_priority():
                            nc.vector.tensor_mul(t1[:], raw[:], cos_sb[:])
                            nc.vector.tensor_mul(t2[:], sw[:], sin_sb[:])
                            rt = pb.tile([128, S], bf16, tag=f"{tag}_r",
                                         bufs=2, name=f"{tag}_roped_{h}")
                            nc.vector.tensor_add(rt[:], t1[:], t2[:])
                        return rt

                    kts, qts = {}, {}

                    def prep(h):
                        kts[h] = load_rope(2 * h, "kr", h)
                        qts[h] = load_rope(2 * h + 1, "qr", h)

                    prep(0)
                    prep(1)
                    for d in range(8):
                        nc.sync.dma_start(
                            out=wo_h0[:, 4 * d:4 * (d + 1), :],
                            in_=woT_v[:, 4 * d:4 * (d + 1), :JC // 2])

                    for h in range(HPC):
                        with nc.named_scope(f"head{h}"):
                            if h + 2 < HPC:
                                prep(h + 2)
                            kT, qT = kts.pop(h), qts.pop(h)
                            hc0 = h * 128
                            attn = pb.tile([128, S], bf16, tag="attn", bufs=2,
                                           name=f"attn_{h}")
                            for g in range(NG):
                                nu = 2 * g + 2      # 1024-wide units
                                nkb = 2 * nu
                                po = psB.tile([128, 512], f32, tag="po",
                                              bufs=2, name=f"po_{h}_{g}")
                                pden = psB.tile([1, 512], f32, tag="pden",
                                                bufs=2, name=f"pden_{h}_{g}")
                                for u in range(nu):
                                    ps2 = psB.tile([128, 1024], f32,
                                                   tag="pss", bufs=2,
                                                   name=f"pss_{h}_{g}_{u}")
                                    diag = u >= 2 * g
                                    for half in range(2):
                                        kb = 2 * u + half
                                        dst = ps2[:, half * 512:
                                                  (half + 1) * 512]
                                        if diag:
                                            mi = 2 * (u - 2 * g) + half
                                            nc.tensor.matmul(
                                                dst, ident[:],
                                                lm_sb[:, mi, :],
                                                start=True, stop=False)
                                        nc.tensor.matmul(
                                            dst,
                                            kT[:, kb * 128:(kb + 1) * 128],
                                            qT[:, g * 512:(g + 1) * 512],
                                            start=not diag, stop=True)
                                    pt = pb.tile([128, 1024], bf16, tag="pt",
                                                 bufs=3,
                                                 name=f"pt_{h}_{g}_{u}")
                                    nc.scalar.activation(pt[:], ps2[:],
                                                         AF.Exp, scale=SCALE)
                                    for half in range(2):
                                        kb = 2 * u + half
                                        ph = pt[:, half * 512:
                                                (half + 1) * 512]
                                        nc.tensor.matmul(
                                            pden[:], ones_b[:], ph,
                                            start=(kb == 0),
                                            stop=(kb == nkb - 1))
                                        nc.tensor.matmul(
                                            po[:],
                                            v_sb[:, kb, hc0:hc0 + 128], ph,
                                            start=(kb == 0),
                                            stop=(kb == nkb - 1))
                                den1 = pb.tile([1, 512], f32, tag="den1",
                                               bufs=2, name=f"den1_{h}_{g}")
                                nc.scalar.copy(den1[:], pden[:])
                                rd1 = pb.tile([1, 512], f32, tag="rd1",
                                              bufs=2, name=f"rd1_{h}_{g}")
                                nc.vector.reciprocal_approx_fast(
                                    out=rd1[:], in_=den1[:])
                                rden = pb.tile([128, 512], f32, tag="rden",
                                               bufs=2, name=f"rden_{h}_{g}")
                                nc.gpsimd.partition_broadcast(rden[:], rd1[:])
                                nc.vector.tensor_mul(
                                    attn[:, g * 512:(g + 1) * 512], po[:],
                                    rden[:])
                            nc.sync.dma_start(out=attn_d[:][h], in_=attn[:])
                            if h % 2 == 1 and h < HPC - 1:
                                p = h // 2
                                nc.gpsimd.collective_compute(
                                    "AllGather", mybir.AluOpType.bypass,
                                    replica_groups=GROUPS,
                                    ins=[attn_d[:][2 * p:2 * p + 2]],
                                    outs=[attn_ag[:][p]])

                # last pair's AllGather outside the stage-B pool scope
                nc.gpsimd.collective_compute(
                    "AllGather", mybir.AluOpType.bypass, replica_groups=GROUPS,
                    ins=[attn_d[:][HPC - 2:HPC]], outs=[attn_ag[:][NPAIR - 1]])

                # ------ stage C: o_proj, pairs 0-2 main + pair-3 increment ---
                with nc.named_scope("stageC"), \
                     tc.tile_pool(name="stC", bufs=1) as pc, \
                     tc.tile_pool(name="psC", bufs=4, space="PSUM") as psC:
                    wo_h1 = pc.tile([128, NIB, JC // 2], bf16, tag="wo1",
                                    bufs=1)
                    for d in range(8):
                        nc.sync.dma_start(
                            out=wo_h1[:, 4 * d:4 * (d + 1), :],
                            in_=woT_v[:, 4 * d:4 * (d + 1), JC // 2:])
                    for tb in range(NKB):
                        at_sb = pc.tile([128, NJB_MAIN, 128], bf16, tag="atC",
                                        bufs=3, name=f"atC_{tb}")
                        nc.sync.dma_start(
                            out=at_sb[:],
                            in_=ag_main[:, :, tb * 128:(tb + 1) * 128])
                        a7_sb = pc.tile([128, NJB_TAIL, 128], bf16, tag="a7C",
                                        bufs=3, name=f"a7C_{tb}")
                        nc.sync.dma_start(
                            out=a7_sb[:],
                            in_=ag_tail[:, :, tb * 128:(tb + 1) * 128])
                        for mc in range(JC // 512):
                            wsrc = wo_h0 if mc == 0 else wo_h1
                            psc = psC.tile([128, 512], f32, tag="psC",
                                           name=f"psC_{tb}_{mc}")
                            for jb in range(NJB_MAIN):
                                nc.tensor.matmul(
                                    psc[:], at_sb[:, jb, :], wsrc[:, jb, :],
                                    start=(jb == 0), stop=(jb == NJB_MAIN - 1))
                            main_sb = pc.tile([128, 512], f32, tag="mainC",
                                              bufs=4, name=f"mainC_{tb}_{mc}")
                            nc.scalar.copy(main_sb[:], psc[:])
                            ps7 = psC.tile([128, 512], f32, tag="ps7", bufs=2,
                                           name=f"ps7_{tb}_{mc}")
                            for i in range(NJB_TAIL):
                                nc.tensor.matmul(
                                    ps7[:], a7_sb[:, i, :],
                                    wsrc[:, NJB_MAIN + i, :],
                                    start=(i == 0), stop=(i == NJB_TAIL - 1))
                            oc = pc.tile([128, 512], f32, tag="oC", bufs=4,
                                         name=f"oC_{tb}_{mc}")
                            nc.vector.tensor_add(oc[:], ps7[:], main_sb[:])
                            nc.sync.dma_start(
                                out=out[:][tb * 128:(tb + 1) * 128,
                                           mc * 512:(mc + 1) * 512],
                                in_=oc[:])

    nc.finalize()
    return nc


_NC_CACHE = None


def _get_nc():
    global _NC_CACHE
    if _NC_CACHE is None:
        _NC_CACHE = build_nc()
    return _NC_CACHE


def _host_inputs(hidden_states, positions, w_pack, w_o):
    hidden_states = np.asarray(hidden_states, dtype=np.float32)
    positions = np.asarray(positions)
    w_pack = np.asarray(w_pack, dtype=np.float32)
    w_o = np.asarray(w_o, dtype=np.float32)

    half = HD // 2
    inv_freq = (1.0 / (THETA ** (np.arange(half, dtype=np.float32) / half)))

    # -1e4 log-mask for the 4 diagonal (128x512) tiles of a q-block
    lmask = np.zeros((4, 128, 512), dtype=np.float32)
    xs = np.arange(512)[None, :]
    ps = np.arange(128)[:, None]
    for v in range(4):
        lmask[v] = np.where(xs >= ps + 128 * v, 0.0, -1e4)

    in_maps = []
    for c in range(NCORES):
        b, r = divmod(c, TPN)
        heads = np.arange(HPC * r, HPC * (r + 1))
        rows = (heads[:, None] * HD + np.arange(HD)[None, :]).reshape(-1)
        Wq = w_pack[rows]                                        # [JC, H]
        Wk = w_pack[H + rows]
        Wv = w_pack[2 * H + rows]
        # qk weight columns interleaved per head: k_h then q_h
        wqk = np.empty((NJT * 128, H), dtype=np.float32)
        for h in range(HPC):
            wqk[256 * h:256 * h + 128] = Wk[128 * h:128 * (h + 1)]
            wqk[256 * h + 128:256 * (h + 1)] = Wq[128 * h:128 * (h + 1)]
        wqk8 = np.ascontiguousarray(wqk.T) * FP8_S               # [H, 2048]
        wvT = np.ascontiguousarray(Wv.T)                         # [H, JC]
        # o_proj m-shard rows, j-order permuted to match the pair-AllGather
        # layout: gathered row (p, r', e, d) holds global head 8*r'+2*p+e
        wo_shard = w_o[JC * r:JC * (r + 1), :]                   # [JC, H]
        woT_full = np.ascontiguousarray(wo_shard.T)              # [H=j, JC]
        woT_perm = woT_full.reshape(TPN, NPAIR, 2, HD, JC) \
                           .transpose(1, 0, 2, 3, 4).reshape(H, JC)
        hsT = np.ascontiguousarray(hidden_states[b].T)           # [H, S]
        ang = positions[b].astype(np.float32)[None, :] * inv_freq[:, None]
        cos_t = np.cos(ang).astype(np.float32)                   # [64, S]
        sin_t = np.sin(ang).astype(np.float32)
        cosf = np.concatenate([cos_t, cos_t], axis=0)            # [128, S]
        sinm = np.concatenate([-sin_t, sin_t], axis=0)
        in_maps.append({
            "hsT": hsT.astype(ml_dtypes.bfloat16),
            "hs8": (hsT * FP8_S).astype(ml_dtypes.float8_e4m3fn),
            "wqk8": wqk8.astype(ml_dtypes.float8_e4m3fn),
            "wvT": wvT.astype(ml_dtypes.bfloat16),
            "woT": np.ascontiguousarray(woT_perm).astype(ml_dtypes.bfloat16),
            "cosf": cosf.astype(ml_dtypes.bfloat16),
            "sinm": sinm.astype(ml_dtypes.bfloat16),
            "lmask": lmask.astype(ml_dtypes.bfloat16),
        })
    return in_maps


def kernel(hidden_states, positions, w_pack, w_o):
    import os
    os.environ["BASS_NEVER_TRACE"] = "1"
    nc = _get_nc()
    in_maps = _host_inputs(hidden_states, positions, w_pack, w_o)
    res = run_bass_kernel_spmd(nc, in_maps, list(range(NCORES)))
    out = np.empty((B, S, H), dtype=np.float32)
    for c in range(NCORES):
        b, r = divmod(c, TPN)
        out[b][:, JC * r:JC * (r + 1)] = res.results[c]["out"]
    return out


# revision 10
# speedup vs baseline: 1.2681x; 1.0340x over previous
"""BaiChuan attention layer on 8 Trainium2 NeuronCores.

Sharding: tensor-parallel over heads within groups of 4 cores (W_pack
column-parallel, o_proj column-parallel after per-head-pair AllGathers
of attention outputs), data-parallel over the batch across the groups.

v3: q/k projection in fp8-e4m3 with DoubleRow matmuls (K=256 per MM,
inputs pre-scaled x64 on host, rescaled 2^-12 on the PSUM-evacuate
copy); v projected in bf16 directly into natural [token, hd] layout
(SBUF-resident, no PE transposes); causal mask applied as a -1e4
PSUM-init via identity matmul (off the DVE); softmax denominator as
accumulated [1,512] ones-matmuls on the PE + reciprocal_approx_fast;
AllGathers batched per head-pair (4 ops) in bf16; o_proj split as
heads 0-5 main + heads 6-7 increment.

Per-core dataflow (core c: batch b=c//4, rank r=c%4, heads 8r..8r+8):
  stage A-qk: for th token-block: qkT[jt] = Wqk_jt @ hs_th   (fp8 DR)
  stage A-v:  for tb 128-token block: v[tb] = hs_tb.T @ Wv   (bf16)
  stage B: per head: neox RoPE on qT,kT (DVE), causal attention with
           sT = kT-blocks.T @ qT, batched exp on ACT, PV from resident
           v, pair AllGathers overlapping later heads' compute.
  stage C: o_proj column-parallel over the gathered head dim.
"""
import sys
sys.path.insert(0, '/opt/trn_rl_repo')
import numpy as np
import ml_dtypes

import concourse.bass as bass
from concourse import bacc
import concourse.mybir as mybir
from concourse.tile import TileContext
from concourse.bass_utils import run_bass_kernel_spmd
from concourse.masks import make_identity

f32 = mybir.dt.float32
bf16 = mybir.dt.bfloat16
fp8 = mybir.dt.float8e4
AF = mybir.ActivationFunctionType
DR = mybir.MatmulPerfMode.DoubleRow

B, S, H, NH = 2, 2048, 4096, 32
HD = H // NH                    # 128
THETA = 10000.0
NCORES, TPN = 8, 4              # 2 groups of 4 (DP over batch x TP over heads)
HPC = NH // TPN                 # 8 heads per core
JC = HPC * HD                   # 1024 per-core q (=k=v) width
SCALE = HD ** -0.5
FP8_S = 64.0                    # host pre-scale on hs8/wqk8
UNSCALE = 1.0 / (FP8_S * FP8_S)  # 2^-12 rescale on the qk PSUM evacuate
GROUPS = [[0, 1, 2, 3], [4, 5, 6, 7]]
NIB = H // 128                  # 32 contraction blocks
NJT = 2 * HPC                   # 16 q/k row-tiles in stage A (k0,q0,k1,q1..)
NG = S // 512                   # 4 query blocks per head
NKB = S // 128                  # 16 key blocks per head
NTB = NKB                       # 16 token 128-blocks
NPAIR = HPC // 2                # 4 AllGather head-pairs
NJB_MAIN = 3 * TPN * 2          # pairs 0-2 -> 24 o_proj jb blocks
NJB_TAIL = TPN * 2              # pair 3 -> 8 jb blocks


def build_nc():
    nc = bacc.Bacc(None)
    hsT = nc.declare_dram_parameter("hsT", [H, S], bf16, isOutput=False)
    hs8 = nc.declare_dram_parameter("hs8", [H, S], fp8, isOutput=False)
    wqk8 = nc.declare_dram_parameter("wqk8", [H, NJT * 128], fp8,
                                     isOutput=False)
    wvT = nc.declare_dram_parameter("wvT", [H, JC], bf16, isOutput=False)
    woT = nc.declare_dram_parameter("woT", [H, JC], bf16, isOutput=False)
    cosf = nc.declare_dram_parameter("cosf", [HD, S], bf16, isOutput=False)
    sinm = nc.declare_dram_parameter("sinm", [HD, S], bf16, isOutput=False)
    lmask = nc.declare_dram_parameter("lmask", [4, 128, 512], bf16,
                                      isOutput=False)
    out = nc.declare_dram_parameter("out", [S, JC], f32, isOutput=True)

    qk_d = [nc.dram_tensor(f"qk_d{j}", [128, S], bf16) for j in range(NJT)]
    attn_d = nc.dram_tensor("attn_d", [HPC, HD, S], bf16)
    attn_ag = nc.dram_tensor("attn_ag", [NPAIR, TPN * 2 * HD, S], bf16)

    hsT_v = hsT[:].rearrange("(n p) t -> p n t", p=128)      # [128, 32, S]
    hs8_v = hs8[:].rearrange("(n p) t -> p n t", p=128)      # [128, 32, S]
    wqk8_v = wqk8[:].rearrange("(n p) j -> p n j", p=128)    # [128, 32, 2048]
    wvT_v = wvT[:].rearrange("(n p) j -> p n j", p=128)      # [128, 32, JC]
    woT_v = woT[:].rearrange("(n p) m -> p n m", p=128)      # [128, 32, JC]
    ag_main = attn_ag[:][0:3].rearrange("p (x q) t -> q (p x) t", q=128)
    ag_tail = attn_ag[:][3].rearrange("(x q) t -> q x t", q=128)

    with TileContext(nc) as tc:
        # persistent pool: resident v (natural layout) + stage-B prep tiles
        with tc.tile_pool(name="P0", bufs=1) as p0:
            v_sb = p0.tile([128, NTB, JC], bf16, tag="v_sb", bufs=1)

            # wv weight pool spans both stage-A passes only
            with tc.tile_pool(name="PAW", bufs=1) as paw:
                wv_sb = paw.tile([128, NIB, JC], bf16, tag="wv", bufs=1)

                # ------------- stage A-qk: q/k projection (fp8 DR) ----------
                with nc.named_scope("stageAqk"), \
                     tc.tile_pool(name="stA", bufs=1) as pa, \
                     tc.tile_pool(name="psA", bufs=6, space="PSUM") as psA:
                    hs_tiles = {}

                    def load_hs8(th):
                        t = pa.tile([128, NIB, 512], fp8, tag="hs", bufs=2,
                                    name=f"hs_{th}")
                        for d in range(4):
                            nc.sync.dma_start(
                                out=t[:, 8 * d:8 * (d + 1), :],
                                in_=hs8_v[:, 8 * d:8 * (d + 1),
                                          th * 512:(th + 1) * 512])
                        hs_tiles[th] = t

                    load_hs8(0)
                    for th in range(S // 512):
                        hs_th = hs_tiles.pop(th)
                        for jt in range(NJT):
                            w_sb = pa.tile([128, NIB, 128], fp8, tag="w",
                                           bufs=4, name=f"w_{th}_{jt}")
                            nc.sync.dma_start(
                                out=w_sb[:],
                                in_=wqk8_v[:, :, jt * 128:(jt + 1) * 128])
                            if jt == 3:
                                # prefetches go behind the first few w loads
                                if th + 1 < S // 512:
                                    load_hs8(th + 1)
                                # wv chunks spread across th iterations
                                for d in (2 * th, 2 * th + 1):
                                    nc.sync.dma_start(
                                        out=wv_sb[:, 4 * d:4 * (d + 1), :],
                                        in_=wvT_v[:, 4 * d:4 * (d + 1), :])
                            ps = psA.tile([128, 512], f32, tag="psA",
                                          name=f"psA_{th}_{jt}")
                            for i2 in range(NIB // 2):
                                nc.tensor.matmul(
                                    ps[:], w_sb[:, 2 * i2:2 * i2 + 2, :],
                                    hs_th[:, 2 * i2:2 * i2 + 2, :],
                                    start=(i2 == 0),
                                    stop=(i2 == NIB // 2 - 1),
                                    perf_mode=DR)
                            st = pa.tile([128, 512], bf16, tag="oA", bufs=4,
                                         name=f"stA_{th}_{jt}")
                            nc.scalar.mul(st[:], ps[:], UNSCALE)
                            nc.sync.dma_start(
                                out=qk_d[jt][:][:, th * 512:(th + 1) * 512],
                                in_=st[:])

                # stage-B prep at P0 level: loads overlap stage A-v
                cos_sb = p0.tile([128, S], bf16, tag="cos", bufs=1)
                sin_sb = p0.tile([128, S], bf16, tag="sin", bufs=1)
                lm_sb = p0.tile([128, 4, 512], bf16, tag="lmask", bufs=1)
                ident = p0.tile([128, 128], bf16, tag="ident", bufs=1)
                ones_b = p0.tile([128, 1], bf16, tag="ones", bufs=1)

                # ------------- stage A-v: v in natural layout (bf16) --------
                with nc.named_scope("stageAv"), \
                     tc.tile_pool(name="stV", bufs=1) as pv, \
                     tc.tile_pool(name="psV", bufs=6, space="PSUM") as psV:
                    hsv_tiles = {}

                    def load_hsv(tb):
                        t = pv.tile([128, NIB, 128], bf16, tag="hsv", bufs=3,
                                    name=f"hsv_{tb}")
                        for d in range(2):
                            nc.sync.dma_start(
                                out=t[:, 16 * d:16 * (d + 1), :],
                                in_=hsT_v[:, 16 * d:16 * (d + 1),
                                          tb * 128:(tb + 1) * 128])
                        hsv_tiles[tb] = t

                    load_hsv(0)
                    load_hsv(1)
                    # prep-tile fills run during stage A-v
                    nc.sync.dma_start(out=cos_sb[:], in_=cosf[:])
                    nc.sync.dma_start(out=sin_sb[:], in_=sinm[:])
                    nc.sync.dma_start(out=lm_sb[:],
                                      in_=lmask[:].rearrange("v p x -> p v x"))
                    make_identity(nc, ident[:])
                    nc.vector.memset(ones_b[:], 1.0)
                    for tb in range(NTB):
                        if tb + 2 < NTB:
                            load_hsv(tb + 2)
                        hs_tb = hsv_tiles.pop(tb)
                        for mc in range(JC // 512):
                            ps = psV.tile([128, 512], f32, tag="psV",
                                          name=f"psV_{tb}_{mc}")
                            for ib in range(NIB):
                                nc.tensor.matmul(
                                    ps[:], hs_tb[:, ib, :],
                                    wv_sb[:, ib, mc * 512:(mc + 1) * 512],
                                    start=(ib == 0), stop=(ib == NIB - 1))
                            nc.vector.tensor_copy(
                                v_sb[:, tb, mc * 512:(mc + 1) * 512], ps[:])

            # ------------- stages B+C share the o_proj weight pool ----------
            with tc.tile_pool(name="stWo", bufs=1, side="right") as pwo:
                wo_h0 = pwo.tile([128, NIB, JC // 2], bf16, tag="wo0", bufs=1)

                # ------------- stage B: rope + causal attention -------------
                wo_h1 = pwo.tile([128, NIB, JC // 2], bf16, tag="wo1",
                                 bufs=1)
                with nc.named_scope("stageB"), \
                     tc.tile_pool(name="stB", bufs=1) as pb, \
                     tc.tile_pool(name="psB", bufs=1, space="PSUM") as psB:

                    def load_rope(jt, tag, h):
                        """load qk_d row-block jt, apply neox rope (bf16)"""
                        raw = pb.tile([128, S], bf16, tag="raw", bufs=8,
                                      name=f"{tag}_raw_{h}")
                        nc.sync.dma_start(out=raw[:], in_=qk_d[jt][:])
                        sw = pb.tile([128, S], bf16, tag="raw", bufs=8,
                                     name=f"{tag}_sw_{h}")
                        nc.sync.dma_start(out=sw[0:64, :],
                                          in_=qk_d[jt][:][64:128, :])
                        nc.sync.dma_start(out=sw[64:128, :],
                                          in_=qk_d[jt][:][0:64, :])
                        t1 = pb.tile([128, S], bf16, tag="ropetmp", bufs=2,
                                     name=f"{tag}_t1_{h}")
                        t2 = pb.tile([128, S], bf16, tag="ropetmp", bufs=2,
                                     name=f"{tag}_t2_{h}")
                        with tc.high_priority():
                            nc.vector.tensor_mul(t1[:], raw[:], cos_sb[:])
                            nc.vector.tensor_mul(t2[:], sw[:], sin_sb[:])
                            rt = pb.tile([128, S], bf16, tag=f"{tag}_r",
                                         bufs=2, name=f"{tag}_roped_{h}")
                            nc.vector.tensor_add(rt[:], t1[:], t2[:])
                        return rt

                    kts, qts = {}, {}

                    def prep(h):
                        kts[h] = load_rope(2 * h, "kr", h)
                        qts[h] = load_rope(2 * h + 1, "qr", h)

                    prep(0)
                    prep(1)
                    for d in range(8):
                        nc.sync.dma_start(
                            out=wo_h0[:, 4 * d:4 * (d + 1), :],
                            in_=woT_v[:, 4 * d:4 * (d + 1), :JC // 2])

                    for h in range(HPC):
                        with nc.named_scope(f"head{h}"):
                            if h + 2 < HPC:
                                prep(h + 2)
                            # wo second half streams in behind the rope loads
                            nc.sync.dma_start(
                                out=wo_h1[:, 4 * h:4 * (h + 1), :],
                                in_=woT_v[:, 4 * h:4 * (h + 1), JC // 2:])
                            kT, qT = kts.pop(h), qts.pop(h)
                            hc0 = h * 128
                            attn = pb.tile([128, S], bf16, tag="attn", bufs=2,
                                           name=f"attn_{h}")
                            for g in range(NG):
                                nu = 2 * g + 2      # 1024-wide units
                                nkb = 2 * nu
                                po = psB.tile([128, 512], f32, tag="po",
                                              bufs=2, name=f"po_{h}_{g}")
                                pden = psB.tile([1, 512], f32, tag="pden",
                                                bufs=2, name=f"pden_{h}_{g}")
                                for u in range(nu):
                                    ps2 = psB.tile([128, 1024], f32,
                                                   tag="pss", bufs=2,
                                                   name=f"pss_{h}_{g}_{u}")
                                    diag = u >= 2 * g
                                    for half in range(2):
                                        kb = 2 * u + half
                                        dst = ps2[:, half * 512:
                                                  (half + 1) * 512]
                                        if diag:
                                            mi = 2 * (u - 2 * g) + half
                                            nc.tensor.matmul(
                                                dst, ident[:],
                                                lm_sb[:, mi, :],
                                                start=True, stop=False)
                                        nc.tensor.matmul(
                                            dst,
                                            kT[:, kb * 128:(kb + 1) * 128],
                                            qT[:, g * 512:(g + 1) * 512],
                                            start=not diag, stop=True)
                                    pt = pb.tile([128, 1024], bf16, tag="pt",
                                                 bufs=3,
                                                 name=f"pt_{h}_{g}_{u}")
                                    nc.scalar.activation(pt[:], ps2[:],
                                                         AF.Exp, scale=SCALE)
                                    for half in range(2):
                                        kb = 2 * u + half
                                        ph = pt[:, half * 512:
                                                (half + 1) * 512]
                                        nc.tensor.matmul(
                                            pden[:], ones_b[:], ph,
                                            start=(kb == 0),
                                            stop=(kb == nkb - 1))
                                        nc.tensor.matmul(
                                            po[:],
                                            v_sb[:, kb, hc0:hc0 + 128], ph,
                                            start=(kb == 0),
                                            stop=(kb == nkb - 1))
                                den1 = pb.tile([1, 512], f32, tag="den1",
                                               bufs=2, name=f"den1_{h}_{g}")
                                nc.scalar.copy(den1[:], pden[:])
                                rd1 = pb.tile([1, 512], f32, tag="rd1",
                                              bufs=2, name=f"rd1_{h}_{g}")
                                nc.vector.reciprocal_approx_fast(
                                    out=rd1[:], in_=den1[:])
                                rden = pb.tile([128, 512], f32, tag="rden",
                                               bufs=2, name=f"rden_{h}_{g}")
                                nc.gpsimd.partition_broadcast(rden[:], rd1[:])
                                nc.vector.tensor_mul(
                                    attn[:, g * 512:(g + 1) * 512], po[:],
                                    rden[:])
                            nc.sync.dma_start(out=attn_d[:][h], in_=attn[:])
                            if h % 2 == 1 and h < HPC - 1:
                                p = h // 2
                                nc.gpsimd.collective_compute(
                                    "AllGather", mybir.AluOpType.bypass,
                                    replica_groups=GROUPS,
                                    ins=[attn_d[:][2 * p:2 * p + 2]],
                                    outs=[attn_ag[:][p]])

                # last pair's AllGather outside the stage-B pool scope
                nc.gpsimd.collective_compute(
                    "AllGather", mybir.AluOpType.bypass, replica_groups=GROUPS,
                    ins=[attn_d[:][HPC - 2:HPC]], outs=[attn_ag[:][NPAIR - 1]])

                # ------ stage C: o_proj, pairs 0-2 main + pair-3 increment ---
                with nc.named_scope("stageC"), \
                     tc.tile_pool(name="stC", bufs=1) as pc, \
                     tc.tile_pool(name="psC", bufs=4, space="PSUM") as psC:
                    for tb in range(NKB):
                        at_sb = pc.tile([128, NJB_MAIN, 128], bf16, tag="atC",
                                        bufs=3, name=f"atC_{tb}")
                        nc.sync.dma_start(
                            out=at_sb[:],
                            in_=ag_main[:, :, tb * 128:(tb + 1) * 128])
                        a7_sb = pc.tile([128, NJB_TAIL, 128], bf16, tag="a7C",
                                        bufs=3, name=f"a7C_{tb}")
                        nc.sync.dma_start(
                            out=a7_sb[:],
                            in_=ag_tail[:, :, tb * 128:(tb + 1) * 128])
                        for mc in range(JC // 512):
                            wsrc = wo_h0 if mc == 0 else wo_h1
                            psc = psC.tile([128, 512], f32, tag="psC",
                                           name=f"psC_{tb}_{mc}")
                            for jb in range(NJB_MAIN):
                                nc.tensor.matmul(
                                    psc[:], at_sb[:, jb, :], wsrc[:, jb, :],
                                    start=(jb == 0), stop=(jb == NJB_MAIN - 1))
                            main_sb = pc.tile([128, 512], f32, tag="mainC",
                                              bufs=4, name=f"mainC_{tb}_{mc}")
                            nc.scalar.copy(main_sb[:], psc[:])
                            ps7 = psC.tile([128, 512], f32, tag="ps7", bufs=2,
                                           name=f"ps7_{tb}_{mc}")
                            for i in range(NJB_TAIL):
                                nc.tensor.matmul(
                                    ps7[:], a7_sb[:, i, :],
                                    wsrc[:, NJB_MAIN + i, :],
                                    start=(i == 0), stop=(i == NJB_TAIL - 1))
                            oc = pc.tile([128, 512], f32, tag="oC", bufs=4,
                                         name=f"oC_{tb}_{mc}")
                            nc.vector.tensor_add(oc[:], ps7[:], main_sb[:])
                            nc.sync.dma_start(
                                out=out[:][tb * 128:(tb + 1) * 128,
                                           mc * 512:(mc + 1) * 512],
                                in_=oc[:])

    nc.finalize()
    return nc


_NC_CACHE = None


def _get_nc():
    global _NC_CACHE
    if _NC_CACHE is None:
        _NC_CACHE = build_nc()
    return _NC_CACHE


def _host_inputs(hidden_states, positions, w_pack, w_o):
    hidden_states = np.asarray(hidden_states, dtype=np.float32)
    positions = np.asarray(positions)
    w_pack = np.asarray(w_pack, dtype=np.float32)
    w_o = np.asarray(w_o, dtype=np.float32)

    half = HD // 2
    inv_freq = (1.0 / (THETA ** (np.arange(half, dtype=np.float32) / half)))

    # -1e4 log-mask for the 4 diagonal (128x512) tiles of a q-block
    lmask = np.zeros((4, 128, 512), dtype=np.float32)
    xs = np.arange(512)[None, :]
    ps = np.arange(128)[:, None]
    for v in range(4):
        lmask[v] = np.where(xs >= ps + 128 * v, 0.0, -1e4)

    in_maps = []
    for c in range(NCORES):
        b, r = divmod(c, TPN)
        heads = np.arange(HPC * r, HPC * (r + 1))
        rows = (heads[:, None] * HD + np.arange(HD)[None, :]).reshape(-1)
        Wq = w_pack[rows]                                        # [JC, H]
        Wk = w_pack[H + rows]
        Wv = w_pack[2 * H + rows]
        # qk weight columns interleaved per head: k_h then q_h
        wqk = np.empty((NJT * 128, H), dtype=np.float32)
        for h in range(HPC):
            wqk[256 * h:256 * h + 128] = Wk[128 * h:128 * (h + 1)]
            wqk[256 * h + 128:256 * (h + 1)] = Wq[128 * h:128 * (h + 1)]
        wqk8 = np.ascontiguousarray(wqk.T) * FP8_S               # [H, 2048]
        wvT = np.ascontiguousarray(Wv.T)                         # [H, JC]
        # o_proj m-shard rows, j-order permuted to match the pair-AllGather
        # layout: gathered row (p, r', e, d) holds global head 8*r'+2*p+e
        wo_shard = w_o[JC * r:JC * (r + 1), :]                   # [JC, H]
        woT_full = np.ascontiguousarray(wo_shard.T)              # [H=j, JC]
        woT_perm = woT_full.reshape(TPN, NPAIR, 2, HD, JC) \
                           .transpose(1, 0, 2, 3, 4).reshape(H, JC)
        hsT = np.ascontiguousarray(hidden_states[b].T)           # [H, S]
        ang = positions[b].astype(np.float32)[None, :] * inv_freq[:, None]
        cos_t = np.cos(ang).astype(np.float32)                   # [64, S]
        sin_t = np.sin(ang).astype(np.float32)
        cosf = np.concatenate([cos_t, cos_t], axis=0)            # [128, S]
        sinm = np.concatenate([-sin_t, sin_t], axis=0)
        in_maps.append({
            "hsT": hsT.astype(ml_dtypes.bfloat16),
            "hs8": (hsT * FP8_S).astype(ml_dtypes.float8_e4m3fn),
            "wqk8": wqk8.astype(ml_dtypes.float8_e4m3fn),
            "wvT": wvT.astype(ml_dtypes.bfloat16),
            "woT": np.ascontiguousarray(woT_perm).astype(ml_dtypes.bfloat16),
            "cosf": cosf.astype(ml_dtypes.bfloat16),
            "sinm": sinm.astype(ml_dtypes.bfloat16),
            "lmask": lmask.astype(ml_dtypes.bfloat16),
        })
    return in_maps


def kernel(hidden_states, positions, w_pack, w_o):
    import os
    os.environ["BASS_NEVER_TRACE"] = "1"
    nc = _get_nc()
    in_maps = _host_inputs(hidden_states, positions, w_pack, w_o)
    res = run_bass_kernel_spmd(nc, in_maps, list(range(NCORES)))
    out = np.empty((B, S, H), dtype=np.float32)
    for c in range(NCORES):
        b, r = divmod(c, TPN)
        out[b][:, JC * r:JC * (r + 1)] = res.results[c]["out"]
    return out
